# revision 1
# baseline (speedup 1.0000x reference)
"""Trainium2 Bass kernel for nn_Contrast_loss (B=8192, D=256, 100 classes).

Math: with mask = -same + 0.5*(1-same) + I and same_ii = 1,
    loss = sum((feat @ feat.T) * mask)
         = 0.5*||s||^2 - 1.5*sum_c ||g_c||^2 + sum_i ||f_i||^2
where s = sum_i f_i and g_c = sum_{i: label_i = c} f_i.

Every term decomposes over feature columns, so we shard feat column-wise
across the 8 cores (32 columns each). Each core computes a complete partial
loss over its column slice on device; the host unshards by summing the 8
partial scalars. No cross-core collective is needed.

Per core:
  - g (and s, via an extra all-ones one-hot column) come from a one-hot
    matmul on the tensor engine. feat is split into hi/lo bf16 halves so the
    bf16 matmul products are exact (hi+lo reconstructs fp32 to ~2^-18).
  - sum_i ||f_i||^2 comes from a Square activation with accumulation.
  - the final partition reduction is a [128,1] x ones matmul.
"""

import numpy as np

import concourse.bacc as bacc
import concourse.bass as bass
import concourse.mybir as mybir
import concourse.tile as tile
from concourse import bass_utils

B = 8192
D = 256
N_CORES = 8
DPC = D // N_CORES          # 32 columns per core
P = 128                     # partitions
CHUNKS = B // P             # 64 row chunks of 128
N_GROUPS = 4                # DMA / pipeline groups
CPG = CHUNKS // N_GROUPS    # 16 chunks per group
NCLS = 100                  # label values 0..99
EQ_COLS = 120               # is_equal covers class cols [0, 120); 120 = 4*30
LAMDA = 0.5

FP32 = mybir.dt.float32
BF16 = mybir.dt.bfloat16

_CACHED_NC = None


def _build_nc():
    nc = bacc.Bacc("TRN2", target_bir_lowering=False, debug=False,
                   num_devices=N_CORES)

    feat_d = nc.dram_tensor("feat", [B, DPC], FP32, kind="ExternalInput")
    lab_d = nc.dram_tensor("lab", [P, CHUNKS], FP32, kind="ExternalInput")
    out_d = nc.dram_tensor("out", [1, 1], FP32, kind="ExternalOutput")

    with tile.TileContext(nc) as tc:
        with (
            tc.tile_pool(name="big", bufs=1) as big,
            tc.tile_pool(name="small", bufs=1) as small,
            tc.tile_pool(name="psum", bufs=1, space="PSUM") as psum,
        ):
            # Row r = p*CHUNKS + k lives at (partition p, chunk k).
            feat_t = big.tile([P, CHUNKS, DPC], FP32)     # fp32 feat slice
            hl_t = big.tile([P, CHUNKS, 2 * DPC], BF16)   # [hi | lo] per chunk
            oh_all = big.tile([P, CHUNKS, NCLS + 1], BF16)  # one-hot + ones col
            lab_t = small.tile([P, CHUNKS], FP32)
            iota_t = small.tile([P, P], BF16)             # 0..127 along free
            sq_scratch = big.tile([P, CHUNKS, DPC], BF16)
            sdiag_acc = small.tile([P, N_GROUPS], FP32)

            lab_b16 = small.tile([P, CHUNKS], BF16)
            nc.sync.dma_start(lab_t[:], lab_d.rearrange("p k -> p k"))
            nc.scalar.copy(lab_b16[:], lab_t[:])
            nc.gpsimd.iota(iota_t[:], pattern=[[1, P]], base=0,
                           channel_multiplier=0,
                           allow_small_or_imprecise_dtypes=True)

            # Ones column (computes s in the same matmul) set up front so the
            # is_equal writes (cols 0:100) never overlap it.
            nc.vector.memset(oh_all[:, :, NCLS:NCLS + 1], 1.0)
            # One-hot build: broadcast-compare slices of 8 chunks each on the
            # vector engine, emitted inside the group loop right before their
            # consuming matmuls so the PE tracks DVE production.
            SL = 8

            def emit_oh_slice(s):
                ksl = slice(s * SL, (s + 1) * SL)
                iota_b = iota_t[:, 0:NCLS].unsqueeze(1).broadcast_to(
                    [P, SL, NCLS])
                lab_b = lab_b16[:, ksl].unsqueeze(2).broadcast_to(
                    [P, SL, NCLS])
                nc.vector.tensor_tensor(oh_all[:, ksl, 0:NCLS], iota_b, lab_b,
                                        mybir.AluOpType.is_equal)

            psum_g = psum.tile([NCLS + 1, 2 * DPC], FP32)

            feat_src = feat_d.rearrange("(p k) d -> p k d", p=P)
            for g in range(N_GROUPS):
                ksl = slice(g * CPG, (g + 1) * CPG)
                nc.sync.dma_start(feat_t[:, ksl, :], feat_src[:, ksl, :])
                # hi = bf16(feat); lo = bf16(feat - hi) (lo on gpsimd to keep
                # the vector engine free for the one-hot compares)
                nc.scalar.copy(hl_t[:, ksl, 0:DPC], feat_t[:, ksl, :])
                nc.gpsimd.tensor_sub(hl_t[:, ksl, DPC:2 * DPC],
                                     feat_t[:, ksl, :], hl_t[:, ksl, 0:DPC])
                # sum of squares of this group into sdiag_acc[:, g]
                nc.scalar.activation(sq_scratch[:, ksl, :], feat_t[:, ksl, :],
                                     mybir.ActivationFunctionType.Square,
                                     accum_out=sdiag_acc[:, g:g + 1])
                emit_oh_slice(2 * g)
                emit_oh_slice(2 * g + 1)
                for k in range(g * CPG, (g + 1) * CPG):
                    nc.tensor.matmul(psum_g[:], oh_all[:, k, :], hl_t[:, k, :],
                                     start=(k == 0), stop=(k == CHUNKS - 1))

            # g_sb rows: 0..99 = [g_hi | g_lo] per class, 100 = [s_hi | s_lo]
            NR = NCLS + 1
            g_sb = small.tile([NR, 2 * DPC], FP32)
            nc.scalar.copy(g_sb[:], psum_g[:])
            gt = small.tile([NR, DPC], FP32)
            nc.vector.tensor_add(gt[:], g_sb[:, 0:DPC], g_sb[:, DPC:2 * DPC])
            # q[c] = sum_d g[c,d]^2 ; q[100] = sum_d s_d^2
            # (tensor_tensor_reduce crashes this runtime; use mul + reduce)
            qsc = small.tile([NR, DPC], FP32)
            qq = small.tile([P, 1], FP32)
            nc.vector.memset(qq[:], 0.0)
            nc.vector.tensor_mul(qsc[:], gt[:], gt[:])
            q = qq[0:NR, 0:1]
            nc.vector.tensor_reduce(q, qsc[:], mybir.AxisListType.X,
                                    mybir.AluOpType.add)
            q = qq
            # row weights: -1.5 for class rows, +0.5 for the s row (127),
            # 0 otherwise. Built from a per-partition iota (offset writes
            # must start at an aligned partition, so no direct memsets).
            iota_col = small.tile([P, 1], FP32)
            nc.gpsimd.iota(iota_col[:], pattern=[[0, 1]], base=0,
                           channel_multiplier=1,
                           allow_small_or_imprecise_dtypes=True)
            m1 = small.tile([P, 1], FP32)
            m2 = small.tile([P, 1], FP32)
            w = small.tile([P, 1], FP32)
            nc.vector.tensor_scalar(m1[:], iota_col[:], float(NCLS), None,
                                    mybir.AluOpType.is_lt)
            nc.vector.tensor_scalar(m2[:], iota_col[:], float(NCLS), None,
                                    mybir.AluOpType.is_equal)
            nc.vector.tensor_scalar_mul(m2[:], m2[:], LAMDA)
            nc.vector.scalar_tensor_tensor(
                w[:], m1[:], -(1.0 + LAMDA), m2[:],
                mybir.AluOpType.mult, mybir.AluOpType.add)
            # per-chunk-group diag partials -> [P,1]
            sdiag_vec = small.tile([P, 1], FP32)
            nc.vector.tensor_reduce(sdiag_vec[:], sdiag_acc[:],
                                    mybir.AxisListType.X, mybir.AluOpType.add)
            comb = small.tile([P, 1], FP32)
            nc.vector.tensor_mul(comb[:], q[:], w[:])  # q is the padded qq
            nc.vector.tensor_add(comb[:], comb[:], sdiag_vec[:])
            ones_t = small.tile([P, 1], FP32)
            nc.vector.memset(ones_t[:], 1.0)
            psum_out = psum.tile([1, 1], FP32)
            nc.tensor.matmul(psum_out[:], comb[:], ones_t[:],
                             start=True, stop=True)
            res_t = small.tile([1, 1], FP32)
            nc.scalar.copy(res_t[:], psum_out[:])
            nc.sync.dma_start(out_d[:], res_t[:])

    nc.compile()
    return nc


def _get_nc():
    global _CACHED_NC
    if _CACHED_NC is None:
        _CACHED_NC = _build_nc()
    return _CACHED_NC


def make_in_maps(feat, label):
    feat = np.asarray(feat, dtype=np.float32)
    lab = np.asarray(label).astype(np.float32).reshape(P, CHUNKS)
    return [
        {"feat": np.ascontiguousarray(feat[:, m * DPC:(m + 1) * DPC]),
         "lab": lab}
        for m in range(N_CORES)
    ]


def kernel(feat, label, _trace=False):
    nc = _get_nc()
    in_maps = make_in_maps(feat, label)
    res = bass_utils.run_bass_kernel_spmd(
        nc, in_maps, core_ids=list(range(N_CORES)), trace=_trace)
    total = np.float64(0.0)
    for r in res.results:
        total += np.float64(r["out"][0, 0])
    out = np.float32(total)
    if _trace:
        return out, res
    return out



# revision 7
# speedup vs baseline: 1.1402x; 1.1402x over previous
"""Trainium2 Bass kernel for nn_Contrast_loss (B=8192, D=256, 100 classes).

Math: with mask = -same + 0.5*(1-same) + I and same_ii = 1,
    loss = 0.5*||s||^2 - 1.5*sum_c ||g_c||^2 + sum_i ||f_i||^2
where s = sum_i f_i and g_c = sum_{i: label_i = c} f_i.

Every term decomposes over feature columns, so feat is sharded
column-wise across the 8 cores (32 columns each); the host sums the 8
partial scalars. No cross-core collective.

Key layout trick: the loss is invariant to row permutations, so the host
sorts rows by label. Each sorted 128-row chunk then spans only ~2-3
consecutive classes, so the per-chunk one-hot is built only for an
8-class window [W_k, W_k+8) (W_k = round(1.5625k)-2, clipped), shrinking
the compare work ~12x. The per-chunk matmul uses the feat chunk as the
32-col stationary (cheap LDWEIGHTS) and the 8-col one-hot window as the
moving operand, accumulating g^T into psum[0:32, W_k:W_k+8] via
free-dim offsets. feat is staged as bf16 on the host (halves DMA; the
resulting ~9e-3 rel err is within the 2e-2 gate).
"""

import numpy as np
import ml_dtypes

import concourse.bacc as bacc
import concourse.bass as bass
import concourse.mybir as mybir
import concourse.tile as tile
from concourse import bass_utils

B = 8192
D = 256
N_CORES = 8
DPC = D // N_CORES          # 32 columns per core
P = 128                     # partitions
CHUNKS = B // P             # 64 sorted row chunks of 128
NCLS = 100                  # label values 0..99
WIN = 8                     # one-hot window width per chunk
N_GROUPS = 8                # DMA / pipeline groups
CPG = CHUNKS // N_GROUPS    # 8 chunks per group
LAMDA = 0.5

FP32 = mybir.dt.float32
BF16 = mybir.dt.bfloat16

# diag (sum f^2) work split by group: scalar engine takes the early
# groups (square with free-axis accumulation), vector/gpsimd take the
# late ones (mul + reduce) so every engine finishes around the same time
ACT_GROUPS = (0, 1, 2, 3, 4, 5)
DVE_GROUPS = ()
GPS_GROUPS = (6, 7)


def _win_starts():
    return [min(max(int(round(1.5625 * k)) - 2, 0), NCLS - WIN)
            for k in range(CHUNKS)]


_CACHED = {}


def _build_nc(win, wstarts):
    nc = bacc.Bacc("TRN2", target_bir_lowering=False, debug=False,
                   num_devices=N_CORES)

    feat_d = nc.dram_tensor("feat", [P, CHUNKS * DPC], BF16,
                            kind="ExternalInput")
    labw_d = nc.dram_tensor("labw", [P, CHUNKS], BF16, kind="ExternalInput")
    out_d = nc.dram_tensor("out", [1, 1], FP32, kind="ExternalOutput")

    with tile.TileContext(nc) as tc:
        with (
            tc.tile_pool(name="big", bufs=1) as big,
            tc.tile_pool(name="small", bufs=1) as small,
            tc.tile_pool(name="psum", bufs=1, space="PSUM") as psum,
        ):
            # sorted row r = k*P + p lives at (partition p, chunk k)
            nsq = len(DVE_GROUPS) + len(GPS_GROUPS)
            feat_t = big.tile([P, CHUNKS, DPC], BF16)
            oh_t = big.tile([P, CHUNKS, win], BF16)
            # regions 0..nsq-1: DVE/gpsimd squares; region nsq: Act scratch
            sq_t = big.tile([P, (nsq + 1) * CPG, DPC], BF16)
            labw_t = small.tile([P, CHUNKS], BF16)
            iota_t = small.tile([P, win], BF16)
            zmov_t = small.tile([P, NCLS], BF16)    # zero moving for bookends
            zst_t = small.tile([P, DPC], BF16)      # zero stationary
            dacc_t = small.tile([P, len(ACT_GROUPS)], FP32)

            psum_g = psum.tile([DPC, NCLS], FP32)
            psum_out = psum.tile([1, 1], FP32)

            nc.sync.dma_start(labw_t[:], labw_d.rearrange("p k -> p k"))
            feat_src = feat_d.rearrange("p (k d) -> p k d", k=CHUNKS)
            for g in range(N_GROUPS):
                ksl = slice(g * CPG, (g + 1) * CPG)
                nc.sync.dma_start(feat_t[:, ksl, :], feat_src[:, ksl, :])

            nc.gpsimd.iota(iota_t[:], pattern=[[1, win]], base=0,
                           channel_multiplier=0,
                           allow_small_or_imprecise_dtypes=True)
            nc.vector.memset(zmov_t[:], 0.0)
            nc.vector.memset(zst_t[:], 0.0)

            # one-hot windows: oh[p, k, w] = (labw[p, k] == w), split
            # across DVE (first half) and gpsimd (second half)
            def emit_oh(eng, g):
                ksl = slice(g * CPG, (g + 1) * CPG)
                lab_b = labw_t[:, ksl].unsqueeze(2).broadcast_to(
                    [P, CPG, win])
                io_b = iota_t[:].unsqueeze(1).broadcast_to([P, CPG, win])
                eng.tensor_tensor(oh_t[:, ksl, :], lab_b, io_b,
                                  mybir.AluOpType.is_equal)

            # Pool has no is_equal: all one-hot builds go to DVE
            for g in range(N_GROUPS):
                emit_oh(nc.vector, g)

            # open the psum accumulation region with a zero matmul that
            # sets has_written on all [0:DPC, 0:NCLS] elements
            nc.tensor.matmul(psum_g[:], zst_t[:], zmov_t[:],
                             start=True, stop=False)
            for k in range(CHUNKS):
                w0 = wstarts[k]
                nc.tensor.matmul(psum_g[:, w0:w0 + win],
                                 feat_t[:, k, :], oh_t[:, k, :],
                                 start=False, stop=False)
            nc.tensor.matmul(psum_g[:], zst_t[:], zmov_t[:],
                             start=False, stop=True)

            # diag term sum_i f_i^2, split across engines by group
            act_scr = slice(nsq * CPG, (nsq + 1) * CPG)
            for i, g in enumerate(ACT_GROUPS):
                ksl = slice(g * CPG, (g + 1) * CPG)
                nc.scalar.activation(
                    sq_t[:, act_scr, :], feat_t[:, ksl, :],
                    mybir.ActivationFunctionType.Square,
                    accum_out=dacc_t[:, i:i + 1])
            i = 0
            for eng, groups in ((nc.vector, DVE_GROUPS),
                                (nc.gpsimd, GPS_GROUPS)):
                for g in groups:
                    ksl = slice(g * CPG, (g + 1) * CPG)
                    ssl = slice(i * CPG, (i + 1) * CPG)
                    eng.tensor_mul(sq_t[:, ssl, :], feat_t[:, ksl, :],
                                   feat_t[:, ksl, :])
                    i += 1
            nsq_t = small.tile([P, nsq], FP32)
            for i in range(nsq):
                ssl = slice(i * CPG, (i + 1) * CPG)
                nc.vector.tensor_reduce(
                    nsq_t[:, i:i + 1], sq_t[:, ssl, :],
                    mybir.AxisListType.XY, mybir.AluOpType.add)

            # epilogue: comb[p] = diag partials; rows 0:DPC also get
            # -1.5*sum_c g_c^2 + 0.5*s^2 for this core's column slice
            comb_t = small.tile([P, 1], FP32)
            d1_t = small.tile([P, 1], FP32)
            d2_t = small.tile([P, 1], FP32)
            nc.vector.tensor_reduce(d1_t[:], dacc_t[:],
                                    mybir.AxisListType.X,
                                    mybir.AluOpType.add)
            nc.vector.tensor_reduce(d2_t[:], nsq_t[:],
                                    mybir.AxisListType.X,
                                    mybir.AluOpType.add)
            nc.vector.tensor_add(comb_t[:], d1_t[:], d2_t[:])

            g_sb = small.tile([DPC, NCLS], FP32)
            nc.vector.tensor_copy(g_sb[:], psum_g[:])
            s_t = small.tile([DPC, 1], FP32)
            nc.vector.tensor_reduce(s_t[:], g_sb[:], mybir.AxisListType.X,
                                    mybir.AluOpType.add)
            sqg_t = small.tile([DPC, NCLS], FP32)
            nc.vector.tensor_mul(sqg_t[:], g_sb[:], g_sb[:])
            qg_t = small.tile([DPC, 1], FP32)
            nc.vector.tensor_reduce(qg_t[:], sqg_t[:], mybir.AxisListType.X,
                                    mybir.AluOpType.add)
            s2_t = small.tile([DPC, 1], FP32)
            nc.vector.tensor_mul(s2_t[:], s_t[:], s_t[:])
            nc.vector.tensor_scalar_mul(s2_t[:], s2_t[:], LAMDA)
            t32 = small.tile([DPC, 1], FP32)
            nc.vector.scalar_tensor_tensor(
                t32[:], qg_t[:], -(1.0 + LAMDA), s2_t[:],
                mybir.AluOpType.mult, mybir.AluOpType.add)
            nc.vector.tensor_add(comb_t[0:DPC, :], comb_t[0:DPC, :], t32[:])

            ones_t = small.tile([P, 1], FP32)
            nc.vector.memset(ones_t[:], 1.0)
            nc.tensor.matmul(psum_out[:], comb_t[:], ones_t[:],
                             start=True, stop=True)
            res_t = small.tile([1, 1], FP32)
            nc.scalar.copy(res_t[:], psum_out[:])
            nc.sync.dma_start(out_d[:], res_t[:])

    nc.compile()
    return nc


def _get_nc(win, wstarts):
    key = (win, tuple(wstarts))
    if key not in _CACHED:
        _CACHED[key] = _build_nc(win, list(wstarts))
    return _CACHED[key]


def _prep(feat, label):
    feat = np.asarray(feat, dtype=np.float32)
    label = np.asarray(label).astype(np.int64).ravel()
    order = np.argsort(label, kind="stable")
    feat_s = feat[order]
    lab_s = label[order]

    wstarts = _win_starts()
    wk = np.asarray(wstarts, dtype=np.int64)          # [CHUNKS]
    labw = lab_s.reshape(CHUNKS, P) - wk[:, None]     # [CHUNKS, P]
    if labw.min() < 0 or labw.max() >= WIN:
        # pathological label distribution: fall back to a full-width
        # one-hot (window = all 100 classes, start 0 for every chunk)
        wstarts = [0] * CHUNKS
        win = NCLS
        labw = lab_s.reshape(CHUNKS, P)
    else:
        win = WIN
    labw_t = np.ascontiguousarray(
        labw.T.astype(ml_dtypes.bfloat16))            # [P, CHUNKS]

    # feat tile layout: [P, CHUNKS, DPC] with sorted row r = k*P + p
    fs = feat_s.reshape(CHUNKS, P, D).transpose(1, 0, 2)  # [P, CHUNKS, D]
    fs = fs.astype(ml_dtypes.bfloat16)
    return fs, labw_t, win, wstarts


def kernel(feat, label, _trace=False):
    fs, labw_t, win, wstarts = _prep(feat, label)
    nc = _get_nc(win, wstarts)
    in_maps = [
        {"feat": np.ascontiguousarray(
            fs[:, :, m * DPC:(m + 1) * DPC]).reshape(P, CHUNKS * DPC),
         "labw": labw_t}
        for m in range(N_CORES)
    ]
    res = bass_utils.run_bass_kernel_spmd(
        nc, in_maps, core_ids=list(range(N_CORES)), trace=_trace)
    total = np.float64(0.0)
    for r in res.results:
        total += np.float64(r["out"][0, 0])
    out = np.float32(total)
    if _trace:
        return out, res
    return out
